# revision 39
# baseline (speedup 1.0000x reference)
"""Trainium2 Bass kernel for nn_Attention_15771119911478 (RBF attention w/ RoPE).

Sharding: core h (of 8) computes head h for both batches (packed on partition
halves). Per-core output is the head's contribution to out @ Wo.T in [s, e]
layout, minus a per-row factor exp(-g*qn[s]) applied on the host. Host sums
the 8 per-core partials.

Host prep per head (cheap O(S*d^2), same spirit as kn/qn in the baseline):
  qro = rope(q @ Wq_h.T).T          [64, S] per batch, bf16
  kro = 2g * rope(q @ Wk_h.T).T     [64, S] per batch, bf16
  vsb = (q @ W_vo) * exp(-g*kn)[:,None]  (w2', the kn bias folded in as a
                                     multiplicative factor; strip-blocked)
Device math per core:
  scs[t,s] = exp(kro[:,t].qro[:,s])              (bias-free exp)
  w2' = vsb (host-computed)                      ( = exp(-g*kn_t) vh Wo_h^T )
  out2[s,e] = sum_t scs[t,s] * w2'[t,e]          (sv flipped: score blocks are
             the stationary operand, w2' streams 64 cols per block)

All PSUM goes through ONE pool tag ([128, 2048] f32 = 4 banks, bufs=2) so
slot reuse is semaphore-based, never a pool-boundary drain. Slot layout is
always b0 in banks 0-1 (cols 0:1024), b1 in banks 2-3 (cols 1024:2048):
a matmul psum write starting at a non-bank-aligned column crashes the device,
and each bank only ever sees one tile_position stream.

A-sweep: strips j=0..7, s in [128j, 1024), one slot per strip, one merged
[128, 2, wA] exp. B-sweep: strips i=0..15, s in [max(1024,128i), 2048), one
slot + one merged exp per strip; sv_i then accumulates into the slot's dead
banks 0/2 (or pre-runs into clean banks 1/3 for late strips) and a 3D copy
evacuates both batches at once.
"""
import os
import sys

sys.path.insert(0, "/opt/trn_rl_repo")

import numpy as np
import ml_dtypes

S = 2048
D = 64
H = 8
B = 2
N_CORES = 8
SCALE = 1.0 / 8.0  # 1/sqrt(64)
BF16 = ml_dtypes.bfloat16

_PROG = None
LAST_RESULTS = None


def _build_program():
    import concourse.bass as bass
    import concourse.bacc as bacc
    import concourse.tile as tile
    from concourse import mybir

    f32 = mybir.dt.float32
    bf16 = mybir.dt.bfloat16
    i32 = mybir.dt.int32
    Exp = mybir.ActivationFunctionType.Exp
    MULT = mybir.AluOpType.mult
    ADD = mybir.AluOpType.add
    FE_A = float(2**23 / np.log(2))
    FE_B = float(127 * 2**23 - 366393)

    nc = bacc.Bacc(
        "TRN2",
        target_bir_lowering=False,
        debug=False,
        enable_asserts=False,
        num_devices=N_CORES,
    )

    def din(name, shape, dt):
        return nc.dram_tensor(name, shape, dt, kind="ExternalInput").ap()

    t_w = din("wcat", [128, 192], bf16)  # wvo|mask
    t_qro = din("qro", [128, S], bf16)
    t_kro = din("kro", [128, S], bf16)
    t_vsb = din("vsb", [128, 2 * 1024], bf16)  # w2' per batch, strip-blocked
    t_out = nc.dram_tensor("out", [128, S], f32, kind="ExternalOutput").ap()

    # strip geometry
    def wA(j):
        return max(0, 1024 - 128 * j)

    def sB(j):
        return max(1024, 128 * j)

    def wB(j):
        return 2048 - sB(j)

    def sc_col(i, j, b):
        # column of s-block i (abs) in scs[j] for batch b
        if 128 * i < 1024:
            return b * wA(j) + 128 * (i - j)
        return 2 * wA(j) + b * wB(j) + 128 * i - sB(j)

    with tile.TileContext(nc) as tc:
        with (
            tc.tile_pool(name="const", bufs=1) as const,
            tc.tile_pool(name="big", bufs=1) as big,
            tc.tile_pool(name="scp", bufs=1) as scp,
            tc.tile_pool(name="pp", bufs=2, space="PSUM") as pp,
        ):
            # ---- SBUF tiles ----
            wcat = const.tile([128, 192], bf16, tag="wcat")
            qro = big.tile([128, S], bf16, tag="qro")
            kro = big.tile([128, S], bf16, tag="kro")
            vsbt = big.tile([128, 2 * 1024], bf16, tag="vsbt")
            fscr = big.tile([128, 2048], f32, tag="fscr")
            vsb = [vsbt[:, 0:1024], vsbt[:, 1024:2048]]
            outsb = big.tile([128, S], f32, tag="outsb")
            scs = {}
            for j in range(16):
                scs[j] = scp.tile(
                    [128, 2 * (2048 - 128 * j)], bf16, tag=f"sc_{j}", name=f"sc_{j}"
                )

            wvo = wcat[:, 0:64]
            mask = wcat[:, 64:192]

            def slot():
                return pp.tile([128, 2048], f32, tag="slot", name="slot")

            # ---- input DMAs: descending-A consumes high columns first ----
            nc.sync.dma_start(wcat[:], t_w[:])
            nc.sync.dma_start(kro[:, 512:1024], t_kro[:, 512:1024])
            nc.sync.dma_start(qro[:, 512:1024], t_qro[:, 512:1024])
            nc.sync.dma_start(kro[:, 0:512], t_kro[:, 0:512])
            nc.sync.dma_start(qro[:, 0:512], t_qro[:, 0:512])
            nc.sync.dma_start(qro[:, 1024:2048], t_qro[:, 1024:2048])
            nc.sync.dma_start(kro[:, 1024:2048], t_kro[:, 1024:2048])
            nc.sync.dma_start(vsbt[:], t_vsb[:])

            # preload ACT exp table (overlaps DMA; wcat lands first)
            scratch = const.tile([128, 1], f32, tag="scratch")
            nc.scalar.activation(scratch[:], wcat[:, 0:1], Exp)

            def qk_mms(dst, b, j, s0, s1, base=0):
                # qk matmuls for strip j, batch b, abs s-range [s0, s1) into
                # psum dst cols [b*1024+base ...); split at 512 bank boundaries
                rows = slice(64 * b, 64 * b + 64)
                tp = (0, 0) if b == 0 else (64, 0)
                off = 0
                while s0 + off < s1:
                    c = base + off
                    wc = min(512 - c % 512, s1 - s0 - off)
                    nc.tensor.matmul(
                        dst[:, b * 1024 + c : b * 1024 + c + wc],
                        kro[rows, j * 128 : j * 128 + 128],
                        qro[rows, s0 + off : s0 + off + wc],
                        start=True, stop=True, tile_position=tp,
                    )
                    off += wc

            def exp3(ps, j, col, w):
                # one merged exp for both batches: [128, 2, w] stride 1024
                in3 = ps.rearrange("p (b c) -> p b c", b=2)[:, :, 0:w]
                out3 = scs[j][:, col : col + 2 * w].rearrange(
                    "p (b c) -> p b c", b=2
                )
                nc.scalar.activation(out3, in3, Exp)

            def emit_A(j, dve=False):
                ps = slot()
                for b in (0, 1):
                    qk_mms(ps, b, j, 128 * j, 1024)
                if dve:
                    # Schraudolph fast-exp on DVE: int32-converting mul-add
                    # writes the float bit pattern of 2^(x*log2e)
                    w = wA(j)
                    in3 = ps.rearrange("p (b c) -> p b c", b=2)[:, :, 0:w]
                    f3 = fscr[:, 0 : 2 * w].rearrange("p (b c) -> p b c", b=2)
                    nc.vector.tensor_scalar(
                        f3.bitcast(i32), in3, FE_A, FE_B, MULT, ADD
                    )
                    nc.vector.tensor_copy(
                        scs[j][:, 0 : 2 * w].rearrange("p (b c) -> p b c", b=2),
                        f3,
                    )
                else:
                    exp3(ps, j, 0, wA(j))
                for b in (0, 1):
                    nc.gpsimd.tensor_mul(
                        scs[j][:, b * wA(j) : b * wA(j) + 128],
                        scs[j][:, b * wA(j) : b * wA(j) + 128],
                        mask[:],
                    )

            bslots = {}

            def emit_qkB(i):
                ps = slot()
                bslots[i] = ps
                for b in (0, 1):
                    qk_mms(ps, b, i, sB(i), 2048)

            def emit_expB(i):
                exp3(bslots[i], i, 2 * wA(i), wB(i))
                if i >= 8:
                    for b in (0, 1):
                        c = sc_col(i, i, b)
                        nc.vector.tensor_mul(
                            scs[i][:, c : c + 128],
                            scs[i][:, c : c + 128],
                            mask[:],
                        )

            def sv_mms(ps, i, pc, js, start_j=0, stop_j=None):
                if stop_j is None:
                    stop_j = i
                for b in (0, 1):
                    for j in js:
                        nc.tensor.matmul(
                            ps[:, b * 1024 + pc : b * 1024 + pc + 64],
                            scs[j][:, sc_col(i, j, b) : sc_col(i, j, b) + 128],
                            vsb[b][:, j * 64 : j * 64 + 64],
                            start=(j == start_j), stop=(j == stop_j),
                        )

            def sv_evac(ps, i, pc):
                pout3 = ps.rearrange("p (b c) -> p b c", b=2)[:, :, pc : pc + 64]
                out3 = outsb[:, 128 * i : 128 * i + 128].rearrange(
                    "p (b c) -> p b c", b=2
                )
                nc.vector.tensor_copy(out3, pout3)
                bslots.pop(i)

            def emit_sv(i):
                # out2 strip i accumulates in dead banks 0/2 of slot i
                sv_mms(bslots[i], i, 0, range(i + 1))
                sv_evac(bslots[i], i, 0)

            def emit_sv_pre(i):
                # strips i>=13: banks 1/3 of slot i are untouched by qk, so
                # blocks j<i can accumulate there before exp_i completes
                sv_mms(bslots[i], i, 512, range(i))

            def emit_sv_post(i):
                sv_mms(bslots[i], i, 512, [i])
                sv_evac(bslots[i], i, 512)

            def emit_A_pair(j1, j2):
                # two strips share one slot: j1 at base 0, j2 at base wA(j1)
                ps = slot()
                for jj, base in ((j1, 0), (j2, wA(j1))):
                    for b in (0, 1):
                        qk_mms(ps, b, jj, 128 * jj, 1024, base=base)
                for jj, base in ((j1, 0), (j2, wA(j1))):
                    in3 = ps.rearrange("p (b c) -> p b c", b=2)[
                        :, :, base : base + wA(jj)
                    ]
                    out3 = scs[jj][:, 0 : 2 * wA(jj)].rearrange(
                        "p (b c) -> p b c", b=2
                    )
                    nc.scalar.activation(out3, in3, Exp)
                    for b in (0, 1):
                        nc.vector.tensor_mul(
                            scs[jj][:, b * wA(jj) : b * wA(jj) + 128],
                            scs[jj][:, b * wA(jj) : b * wA(jj) + 128],
                            mask[:],
                        )

            # ---- emission order: descending-width A (long exps last so B0's
            # qk hides under them); v in two waves mid-A, one shared slot ----
            emit_A(7, dve=True)
            emit_A(6, dve=True)
            emit_A(5, dve=True)
            emit_A(4, dve=True)
            emit_A(3, dve=True)
            emit_A(2)
            emit_A(1, dve=True)
            emit_A(0)

            emit_qkB(0)
            emit_expB(0)
            for i in range(1, 12):
                emit_qkB(i)
                emit_sv(i - 1)
                emit_expB(i)
                if i % 4 == 0:
                    k = i // 4 - 1
                    nc.sync.dma_start(
                        t_out[:, 512 * k : 512 * k + 512],
                        outsb[:, 512 * k : 512 * k + 512],
                    )
            # strips 12-15: two strips per slot (wB <= 512); qk for the pair
            # lands before either exp; sv pre-runs into the slot's free cols
            pc_of = {12: 0, 13: 512, 14: 0, 15: 512}
            for a in (12, 14):
                ps = slot()
                for ii in (a, a + 1):
                    bslots[ii] = ps
                    for b in (0, 1):
                        qk_mms(ps, b, ii, sB(ii), 2048, base=pc_of[ii])
                if a == 12:
                    emit_sv(11)
                for ii in (a, a + 1):
                    in3 = bslots[ii].rearrange("p (b c) -> p b c", b=2)[
                        :, :, pc_of[ii] : pc_of[ii] + wB(ii)
                    ]
                    out3 = scs[ii][:, 0 : 2 * wB(ii)].rearrange(
                        "p (b c) -> p b c", b=2
                    )
                    nc.scalar.activation(out3, in3, Exp)
                    for b in (0, 1):
                        c = sc_col(ii, ii, b)
                        nc.vector.tensor_mul(
                            scs[ii][:, c : c + 128],
                            scs[ii][:, c : c + 128],
                            mask[:],
                        )
                if a == 12:
                    nc.sync.dma_start(t_out[:, 1024:1536], outsb[:, 1024:1536])
            # tail: interleave sv chains across the two pair tiles so each
            # chain's evac (DVE) hides under the other tile's sv matmuls
            ps12, ps14 = bslots[12], bslots[14]
            sv_mms(ps12, 12, 0, range(13))
            sv_evac(ps12, 12, 0)
            sv_mms(ps14, 14, 0, range(15))
            sv_evac(ps14, 14, 0)
            sv_mms(ps12, 13, 512, range(14))
            sv_evac(ps12, 13, 512)
            nc.sync.dma_start(t_out[:, 1536:1920], outsb[:, 1536:1920])
            sv_mms(ps14, 15, 512, range(16))
            sv_evac(ps14, 15, 512)
            nc.sync.dma_start(t_out[:, 1920:2048], outsb[:, 1920:2048])

    nc.compile()
    return nc


def _get_program():
    global _PROG
    if _PROG is None:
        _PROG = _build_program()
    return _PROG


def _rope_T(x):
    # interleaved RoPE on [S, 64], returns [64, S] f32
    f = np.arange(32, dtype=np.float64)
    freqs = 1.0 / (10000.0 ** (2 * f / 64))
    ang = np.arange(S, dtype=np.float64)[:, None] * freqs[None, :]
    c = np.cos(ang)
    s = np.sin(ang)
    x1, x2 = x[:, 0::2].astype(np.float64), x[:, 1::2].astype(np.float64)
    out = np.empty((S, 64), np.float64)
    out[:, 0::2] = x1 * c - x2 * s
    out[:, 1::2] = x1 * s + x2 * c
    return out.T.astype(np.float32)


def _prep_inputs(q, Wq, Wk, Wv, Wo, gamma):
    """Build the per-core in_maps (all host-side numpy)."""
    q = np.asarray(q, np.float32)
    Wq = np.asarray(Wq, np.float32)
    Wk = np.asarray(Wk, np.float32)
    Wv = np.asarray(Wv, np.float32)
    Wo = np.asarray(Wo, np.float32)
    gamma = np.asarray(gamma, np.float32)

    mask = np.triu(np.ones((128, 128), np.float32)).astype(BF16)

    def dup(x):
        return np.concatenate([x, x], 0)

    in_maps = []
    qn_exp = np.zeros((B, H, S), np.float32)
    for h in range(H):
        g = float(gamma[h]) * SCALE
        Wq_h = Wq[h * 64 : (h + 1) * 64]
        Wk_h = Wk[h * 64 : (h + 1) * 64]
        Wv_h = Wv[h * 64 : (h + 1) * 64]
        Wo_h = Wo[:, h * 64 : (h + 1) * 64]  # [64(e), 64(d)]
        W_vo = Wv_h.T @ Wo_h.T  # [64(i), 64(e)] : q @ W_vo = vh @ Wo_h.T

        qro_b, kro_b, vsb_b = [], [], []
        for b in range(B):
            qh = q[b] @ Wq_h.T
            kh = q[b] @ Wk_h.T
            qro_b.append(_rope_T(qh))
            kro_b.append(_rope_T(kh) * (2.0 * g))
            kn = (kh * kh).sum(-1)
            w2 = (q[b] @ W_vo) * np.exp(-g * kn)[:, None]  # [S, 64]
            vsb_b.append(
                w2.reshape(16, 128, 64).transpose(1, 0, 2).reshape(128, 1024)
            )
            qn = (qh * qh).sum(-1)
            qn_exp[b, h] = np.exp(-g * qn)

        qro = np.concatenate(qro_b, 0).astype(BF16)  # [128, S]
        kro = np.concatenate(kro_b, 0).astype(BF16)
        vsb = np.concatenate(vsb_b, 1).astype(BF16)  # [128, 2*1024]
        wcat = np.concatenate([dup(W_vo).astype(BF16), mask], axis=1)

        in_maps.append(
            {
                "wcat": np.ascontiguousarray(wcat),
                "qro": np.ascontiguousarray(qro),
                "kro": np.ascontiguousarray(kro),
                "vsb": np.ascontiguousarray(vsb),
            }
        )
    return in_maps, qn_exp


def kernel(q, Wq, Wk, Wv, Wo, gamma):
    global LAST_RESULTS
    from concourse import bass_utils

    nc = _get_program()
    in_maps, qn_exp = _prep_inputs(q, Wq, Wk, Wv, Wo, gamma)
    trace = bool(int(os.environ.get("KERNEL_TRACE", "0")))
    res = bass_utils.run_bass_kernel_spmd(
        nc, in_maps, core_ids=list(range(N_CORES)), trace=trace
    )
    LAST_RESULTS = res

    final = np.zeros((B, S, D), np.float32)
    for h in range(H):
        o = np.asarray(res.results[h]["out"], np.float32)  # [128, S]
        # col block i: [b0(64) | b1(64)] for s-strip i; row r = s offset
        o4 = o.reshape(128, 16, 2, 64)  # [r, i, b, e]
        for b in range(B):
            ob = o4[:, :, b, :].transpose(1, 0, 2).reshape(S, D)  # [s, e]
            final[b] += ob * qn_exp[b, h][:, None]
    return final


# revision 40
# speedup vs baseline: 1.0632x; 1.0632x over previous
"""Trainium2 Bass kernel for nn_Attention_15771119911478 (RBF attention w/ RoPE).

Sharding: core h (of 8) computes head h for both batches (packed on partition
halves). Per-core output is the head's contribution to out @ Wo.T in [s, e]
layout, minus a per-row factor exp(-g*qn[s]) applied on the host. Host sums
the 8 per-core partials.

Host prep per head (cheap O(S*d^2), same spirit as kn/qn in the baseline):
  qro = rope(q @ Wq_h.T).T          [64, S] per batch, bf16
  kro = 2g * rope(q @ Wk_h.T).T     [64, S] per batch, bf16
  vsb = (q @ W_vo) * exp(-g*kn)[:,None]  (w2', the kn bias folded in as a
                                     multiplicative factor; strip-blocked)
Device math per core:
  scs[t,s] = exp(kro[:,t].qro[:,s])              (bias-free exp)
  w2' = vsb (host-computed)                      ( = exp(-g*kn_t) vh Wo_h^T )
  out2[s,e] = sum_t scs[t,s] * w2'[t,e]          (sv flipped: score blocks are
             the stationary operand, w2' streams 64 cols per block)

All PSUM goes through ONE pool tag ([128, 2048] f32 = 4 banks, bufs=2) so
slot reuse is semaphore-based, never a pool-boundary drain. Slot layout is
always b0 in banks 0-1 (cols 0:1024), b1 in banks 2-3 (cols 1024:2048):
a matmul psum write starting at a non-bank-aligned column crashes the device,
and each bank only ever sees one tile_position stream.

A-sweep: strips j=0..7, s in [128j, 1024), one slot per strip, one merged
[128, 2, wA] exp. B-sweep: strips i=0..15, s in [max(1024,128i), 2048), one
slot + one merged exp per strip; sv_i then accumulates into the slot's dead
banks 0/2 (or pre-runs into clean banks 1/3 for late strips) and a 3D copy
evacuates both batches at once.
"""
import os
import sys

sys.path.insert(0, "/opt/trn_rl_repo")

import numpy as np
import ml_dtypes

S = 2048
D = 64
H = 8
B = 2
N_CORES = 8
SCALE = 1.0 / 8.0  # 1/sqrt(64)
BF16 = ml_dtypes.bfloat16

_PROG = None
LAST_RESULTS = None


def _build_program():
    import concourse.bass as bass
    import concourse.bacc as bacc
    import concourse.tile as tile
    from concourse import mybir

    f32 = mybir.dt.float32
    bf16 = mybir.dt.bfloat16
    i32 = mybir.dt.int32
    Exp = mybir.ActivationFunctionType.Exp
    MULT = mybir.AluOpType.mult
    ADD = mybir.AluOpType.add
    FE_A = float(2**23 / np.log(2))
    FE_B = float(127 * 2**23 - 366393)

    nc = bacc.Bacc(
        "TRN2",
        target_bir_lowering=False,
        debug=False,
        enable_asserts=False,
        num_devices=N_CORES,
    )

    def din(name, shape, dt):
        return nc.dram_tensor(name, shape, dt, kind="ExternalInput").ap()

    t_w = din("wcat", [128, 192], bf16)  # wvo|mask
    t_qro = din("qro", [128, S], bf16)
    t_kro = din("kro", [128, S], bf16)
    t_vsb = din("vsb", [128, 2 * 1024], bf16)  # w2' per batch, strip-blocked
    t_out = nc.dram_tensor("out", [128, S], f32, kind="ExternalOutput").ap()

    # strip geometry
    def wA(j):
        return max(0, 1024 - 128 * j)

    def sB(j):
        return max(1024, 128 * j)

    def wB(j):
        return 2048 - sB(j)

    def sc_col(i, j, b):
        # column of s-block i (abs) in scs[j] for batch b
        if 128 * i < 1024:
            return b * wA(j) + 128 * (i - j)
        return 2 * wA(j) + b * wB(j) + 128 * i - sB(j)

    with tile.TileContext(nc) as tc:
        with (
            tc.tile_pool(name="const", bufs=1) as const,
            tc.tile_pool(name="big", bufs=1) as big,
            tc.tile_pool(name="scp", bufs=1) as scp,
            tc.tile_pool(name="pp", bufs=2, space="PSUM") as pp,
        ):
            # ---- SBUF tiles ----
            wcat = const.tile([128, 192], bf16, tag="wcat")
            qro = big.tile([128, S], bf16, tag="qro")
            kro = big.tile([128, S], bf16, tag="kro")
            vsbt = big.tile([128, 2 * 1024], bf16, tag="vsbt")
            fscr = big.tile([128, 2048], f32, tag="fscr")
            vsb = [vsbt[:, 0:1024], vsbt[:, 1024:2048]]
            outsb = big.tile([128, S], f32, tag="outsb")
            scs = {}
            for j in range(16):
                scs[j] = scp.tile(
                    [128, 2 * (2048 - 128 * j)], bf16, tag=f"sc_{j}", name=f"sc_{j}"
                )

            wvo = wcat[:, 0:64]
            mask = wcat[:, 64:192]

            def slot():
                return pp.tile([128, 2048], f32, tag="slot", name="slot")

            # ---- input DMAs: descending-A consumes high columns first ----
            nc.sync.dma_start(wcat[:], t_w[:])
            nc.sync.dma_start(kro[:, 512:1024], t_kro[:, 512:1024])
            nc.sync.dma_start(qro[:, 512:1024], t_qro[:, 512:1024])
            nc.sync.dma_start(kro[:, 0:512], t_kro[:, 0:512])
            nc.sync.dma_start(qro[:, 0:512], t_qro[:, 0:512])
            nc.sync.dma_start(qro[:, 1024:2048], t_qro[:, 1024:2048])
            nc.sync.dma_start(kro[:, 1024:2048], t_kro[:, 1024:2048])
            nc.sync.dma_start(vsbt[:], t_vsb[:])

            # preload ACT exp table (overlaps DMA; wcat lands first)
            scratch = const.tile([128, 1], f32, tag="scratch")
            nc.scalar.activation(scratch[:], wcat[:, 0:1], Exp)

            def qk_mms(dst, b, j, s0, s1, base=0):
                # qk matmuls for strip j, batch b, abs s-range [s0, s1) into
                # psum dst cols [b*1024+base ...); split at 512 bank boundaries
                rows = slice(64 * b, 64 * b + 64)
                tp = (0, 0) if b == 0 else (64, 0)
                off = 0
                while s0 + off < s1:
                    c = base + off
                    wc = min(512 - c % 512, s1 - s0 - off)
                    nc.tensor.matmul(
                        dst[:, b * 1024 + c : b * 1024 + c + wc],
                        kro[rows, j * 128 : j * 128 + 128],
                        qro[rows, s0 + off : s0 + off + wc],
                        start=True, stop=True, tile_position=tp,
                    )
                    off += wc

            def exp3(ps, j, col, w):
                # one merged exp for both batches: [128, 2, w] stride 1024
                in3 = ps.rearrange("p (b c) -> p b c", b=2)[:, :, 0:w]
                out3 = scs[j][:, col : col + 2 * w].rearrange(
                    "p (b c) -> p b c", b=2
                )
                nc.scalar.activation(out3, in3, Exp)

            def emit_A(j, dve=False):
                ps = slot()
                for b in (0, 1):
                    qk_mms(ps, b, j, 128 * j, 1024)
                if dve:
                    # Schraudolph fast-exp on DVE: int32-converting mul-add
                    # writes the float bit pattern of 2^(x*log2e)
                    w = wA(j)
                    in3 = ps.rearrange("p (b c) -> p b c", b=2)[:, :, 0:w]
                    f3 = fscr[:, 0 : 2 * w].rearrange("p (b c) -> p b c", b=2)
                    nc.vector.tensor_scalar(
                        f3.bitcast(i32), in3, FE_A, FE_B, MULT, ADD
                    )
                    nc.vector.tensor_copy(
                        scs[j][:, 0 : 2 * w].rearrange("p (b c) -> p b c", b=2),
                        f3,
                    )
                else:
                    exp3(ps, j, 0, wA(j))
                for b in (0, 1):
                    nc.gpsimd.tensor_mul(
                        scs[j][:, b * wA(j) : b * wA(j) + 128],
                        scs[j][:, b * wA(j) : b * wA(j) + 128],
                        mask[:],
                    )

            bslots = {}

            def emit_qkB(i):
                ps = slot()
                bslots[i] = ps
                for b in (0, 1):
                    qk_mms(ps, b, i, sB(i), 2048)

            def emit_expB(i):
                exp3(bslots[i], i, 2 * wA(i), wB(i))
                if i >= 8:
                    for b in (0, 1):
                        c = sc_col(i, i, b)
                        nc.vector.tensor_mul(
                            scs[i][:, c : c + 128],
                            scs[i][:, c : c + 128],
                            mask[:],
                        )

            def sv_mms(ps, i, pc, js, start_j=0, stop_j=None):
                if stop_j is None:
                    stop_j = i
                for b in (0, 1):
                    for j in js:
                        nc.tensor.matmul(
                            ps[:, b * 1024 + pc : b * 1024 + pc + 64],
                            scs[j][:, sc_col(i, j, b) : sc_col(i, j, b) + 128],
                            vsb[b][:, j * 64 : j * 64 + 64],
                            start=(j == start_j), stop=(j == stop_j),
                        )

            def sv_evac(ps, i, pc):
                pout3 = ps.rearrange("p (b c) -> p b c", b=2)[:, :, pc : pc + 64]
                out3 = outsb[:, 128 * i : 128 * i + 128].rearrange(
                    "p (b c) -> p b c", b=2
                )
                nc.vector.tensor_copy(out3, pout3)
                bslots.pop(i)

            def emit_sv(i):
                # out2 strip i accumulates in dead banks 0/2 of slot i
                sv_mms(bslots[i], i, 0, range(i + 1))
                sv_evac(bslots[i], i, 0)

            def emit_sv_pre(i):
                # strips i>=13: banks 1/3 of slot i are untouched by qk, so
                # blocks j<i can accumulate there before exp_i completes
                sv_mms(bslots[i], i, 512, range(i))

            def emit_sv_post(i):
                sv_mms(bslots[i], i, 512, [i])
                sv_evac(bslots[i], i, 512)

            def emit_A_pair(j1, j2):
                # two strips share one slot: j1 at base 0, j2 at base wA(j1)
                ps = slot()
                for jj, base in ((j1, 0), (j2, wA(j1))):
                    for b in (0, 1):
                        qk_mms(ps, b, jj, 128 * jj, 1024, base=base)
                for jj, base in ((j1, 0), (j2, wA(j1))):
                    in3 = ps.rearrange("p (b c) -> p b c", b=2)[
                        :, :, base : base + wA(jj)
                    ]
                    out3 = scs[jj][:, 0 : 2 * wA(jj)].rearrange(
                        "p (b c) -> p b c", b=2
                    )
                    nc.scalar.activation(out3, in3, Exp)
                    for b in (0, 1):
                        nc.vector.tensor_mul(
                            scs[jj][:, b * wA(jj) : b * wA(jj) + 128],
                            scs[jj][:, b * wA(jj) : b * wA(jj) + 128],
                            mask[:],
                        )

            # ---- emission order: descending-width A (long exps last so B0's
            # qk hides under them); v in two waves mid-A, one shared slot ----
            emit_A(7, dve=True)
            emit_A(6)
            emit_A(5, dve=True)
            emit_A(4)
            emit_A(3, dve=True)
            emit_A(2)
            emit_A(1, dve=True)
            emit_A(0)

            emit_qkB(0)
            emit_expB(0)
            for i in range(1, 12):
                emit_qkB(i)
                emit_sv(i - 1)
                emit_expB(i)
                if i % 4 == 0:
                    k = i // 4 - 1
                    nc.sync.dma_start(
                        t_out[:, 512 * k : 512 * k + 512],
                        outsb[:, 512 * k : 512 * k + 512],
                    )
            # strips 12-15: two strips per slot (wB <= 512); qk for the pair
            # lands before either exp; sv pre-runs into the slot's free cols
            pc_of = {12: 0, 13: 512, 14: 0, 15: 512}
            for a in (12, 14):
                ps = slot()
                for ii in (a, a + 1):
                    bslots[ii] = ps
                    for b in (0, 1):
                        qk_mms(ps, b, ii, sB(ii), 2048, base=pc_of[ii])
                if a == 12:
                    emit_sv(11)
                for ii in (a, a + 1):
                    in3 = bslots[ii].rearrange("p (b c) -> p b c", b=2)[
                        :, :, pc_of[ii] : pc_of[ii] + wB(ii)
                    ]
                    out3 = scs[ii][:, 0 : 2 * wB(ii)].rearrange(
                        "p (b c) -> p b c", b=2
                    )
                    nc.scalar.activation(out3, in3, Exp)
                    for b in (0, 1):
                        c = sc_col(ii, ii, b)
                        nc.vector.tensor_mul(
                            scs[ii][:, c : c + 128],
                            scs[ii][:, c : c + 128],
                            mask[:],
                        )
                if a == 12:
                    nc.sync.dma_start(t_out[:, 1024:1536], outsb[:, 1024:1536])
            # tail: interleave sv chains across the two pair tiles so each
            # chain's evac (DVE) hides under the other tile's sv matmuls
            ps12, ps14 = bslots[12], bslots[14]
            sv_mms(ps12, 12, 0, range(13))
            sv_evac(ps12, 12, 0)
            sv_mms(ps14, 14, 0, range(15))
            sv_evac(ps14, 14, 0)
            sv_mms(ps12, 13, 512, range(14))
            sv_evac(ps12, 13, 512)
            nc.sync.dma_start(t_out[:, 1536:1920], outsb[:, 1536:1920])
            sv_mms(ps14, 15, 512, range(16))
            sv_evac(ps14, 15, 512)
            nc.sync.dma_start(t_out[:, 1920:2048], outsb[:, 1920:2048])

    nc.compile()
    return nc


def _get_program():
    global _PROG
    if _PROG is None:
        _PROG = _build_program()
    return _PROG


def _rope_T(x):
    # interleaved RoPE on [S, 64], returns [64, S] f32
    f = np.arange(32, dtype=np.float64)
    freqs = 1.0 / (10000.0 ** (2 * f / 64))
    ang = np.arange(S, dtype=np.float64)[:, None] * freqs[None, :]
    c = np.cos(ang)
    s = np.sin(ang)
    x1, x2 = x[:, 0::2].astype(np.float64), x[:, 1::2].astype(np.float64)
    out = np.empty((S, 64), np.float64)
    out[:, 0::2] = x1 * c - x2 * s
    out[:, 1::2] = x1 * s + x2 * c
    return out.T.astype(np.float32)


def _prep_inputs(q, Wq, Wk, Wv, Wo, gamma):
    """Build the per-core in_maps (all host-side numpy)."""
    q = np.asarray(q, np.float32)
    Wq = np.asarray(Wq, np.float32)
    Wk = np.asarray(Wk, np.float32)
    Wv = np.asarray(Wv, np.float32)
    Wo = np.asarray(Wo, np.float32)
    gamma = np.asarray(gamma, np.float32)

    mask = np.triu(np.ones((128, 128), np.float32)).astype(BF16)

    def dup(x):
        return np.concatenate([x, x], 0)

    in_maps = []
    qn_exp = np.zeros((B, H, S), np.float32)
    for h in range(H):
        g = float(gamma[h]) * SCALE
        Wq_h = Wq[h * 64 : (h + 1) * 64]
        Wk_h = Wk[h * 64 : (h + 1) * 64]
        Wv_h = Wv[h * 64 : (h + 1) * 64]
        Wo_h = Wo[:, h * 64 : (h + 1) * 64]  # [64(e), 64(d)]
        W_vo = Wv_h.T @ Wo_h.T  # [64(i), 64(e)] : q @ W_vo = vh @ Wo_h.T

        qro_b, kro_b, vsb_b = [], [], []
        for b in range(B):
            qh = q[b] @ Wq_h.T
            kh = q[b] @ Wk_h.T
            qro_b.append(_rope_T(qh))
            kro_b.append(_rope_T(kh) * (2.0 * g))
            kn = (kh * kh).sum(-1)
            w2 = (q[b] @ W_vo) * np.exp(-g * kn)[:, None]  # [S, 64]
            vsb_b.append(
                w2.reshape(16, 128, 64).transpose(1, 0, 2).reshape(128, 1024)
            )
            qn = (qh * qh).sum(-1)
            qn_exp[b, h] = np.exp(-g * qn)

        qro = np.concatenate(qro_b, 0).astype(BF16)  # [128, S]
        kro = np.concatenate(kro_b, 0).astype(BF16)
        vsb = np.concatenate(vsb_b, 1).astype(BF16)  # [128, 2*1024]
        wcat = np.concatenate([dup(W_vo).astype(BF16), mask], axis=1)

        in_maps.append(
            {
                "wcat": np.ascontiguousarray(wcat),
                "qro": np.ascontiguousarray(qro),
                "kro": np.ascontiguousarray(kro),
                "vsb": np.ascontiguousarray(vsb),
            }
        )
    return in_maps, qn_exp


def kernel(q, Wq, Wk, Wv, Wo, gamma):
    global LAST_RESULTS
    from concourse import bass_utils

    nc = _get_program()
    in_maps, qn_exp = _prep_inputs(q, Wq, Wk, Wv, Wo, gamma)
    trace = bool(int(os.environ.get("KERNEL_TRACE", "0")))
    res = bass_utils.run_bass_kernel_spmd(
        nc, in_maps, core_ids=list(range(N_CORES)), trace=trace
    )
    LAST_RESULTS = res

    final = np.zeros((B, S, D), np.float32)
    for h in range(H):
        o = np.asarray(res.results[h]["out"], np.float32)  # [128, S]
        # col block i: [b0(64) | b1(64)] for s-strip i; row r = s offset
        o4 = o.reshape(128, 16, 2, 64)  # [r, i, b, e]
        for b in range(B):
            ob = o4[:, :, b, :].transpose(1, 0, 2).reshape(S, D)  # [s, e]
            final[b] += ob * qn_exp[b, h][:, None]
    return final
